# revision 4
# baseline (speedup 1.0000x reference)
"""Trainium2 Bass kernel for nn_ActorSpine (population-coding encoder MLP actor).

Reference computation (per sample):
  spine = sigmoid((state[:, :, None] - mean_enc) / std_enc)  # [B, 128, 10]
  a1 = relu(spine.reshape(B, 1280) @ W1.T + b1)              # [B, 2048]
  a2 = relu(a1 @ W2.T + b2)                                  # [B, 2048]
  a3 = a2 @ W3.T + b3                                        # [B, 320]
  raw = einsum('bak,ak->ba', a3.reshape(B, 32, 10), Wd[:, 0]) + bd
  out = tanh(raw)                                            # [B, 32]

Strategy: pure data parallel over 8 cores (2048 samples each).
Host-side folding:
  - decoder conv folds into W3: W3p[a, h] = sum_k Wd[a,0,k] * W3[a*10+k, h],
    b3p[a] = sum_k Wd[a,0,k]*b3[a*10+k] + bd[a]  -> final layer is [32, 2048]
  - encoder contraction index permuted j' = k*128 + d so spine k-tiles are
    plain per-partition sigmoid activations of stateT; W1 columns permuted to
    match.
Device: activations kept transposed [feature, batch]; fp16 matmul inputs
(full TensorE rate), fp32 PSUM accumulation, ScalarE fused bias+relu/tanh.
"""

import numpy as np

import concourse.bass as bass
import concourse.mybir as mybir
import concourse.tile as tile
from concourse import bacc
from concourse.bass_utils import run_bass_kernel_spmd

# Problem dims (hardcoded per harness contract)
B = 16384
D = 128
ENC_K = 10
ACT_DIM = 32
DEC_K = 10
H0 = 2048
H1 = 2048
NCORES = 8
BL = B // NCORES  # 2048 samples per core
NT = 512          # moving-dim tile (one PSUM bank of fp32)
NSUB = BL // NT   # 4
M1 = H0 // 128    # 16 m-tiles for layer 1
K1 = ENC_K        # 10 k-tiles for layer 1 (permuted encoder)
M2 = H1 // 128    # 16
K2 = H0 // 128    # 16
K3 = H1 // 128    # 16

F16 = mybir.dt.float16
F32 = mybir.dt.float32

_cached = {}


def _build_program():
    if "nc" in _cached:
        return _cached["nc"]

    nc = bacc.Bacc("TRN2", target_bir_lowering=False, debug=False,
                   num_devices=NCORES)

    stateT = nc.dram_tensor("stateT", [D, BL], F32, kind="ExternalInput")
    w1t = nc.dram_tensor("w1t", [M1, 128, K1, 128], F16, kind="ExternalInput")
    w2t = nc.dram_tensor("w2t", [M2, 128, K2, 128], F16, kind="ExternalInput")
    w3t = nc.dram_tensor("w3t", [128, K3, ACT_DIM], F16, kind="ExternalInput")
    # scalars layout (per partition p): [0:10] enc_scale, [10:20] enc_bias,
    # [20:36] b1, [36:52] b2, [52] b3p (partitions 0..31)
    scal = nc.dram_tensor("scal", [128, 53], F32, kind="ExternalInput")
    out = nc.dram_tensor("out", [ACT_DIM, BL], F32, kind="ExternalOutput")

    with tile.TileContext(nc) as tc:
        with (
            tc.tile_pool(name="consts", bufs=1) as consts,
            tc.tile_pool(name="acts", bufs=1) as acts,
            tc.tile_pool(name="h2p", bufs=4) as h2p,
            tc.tile_pool(name="w1p", bufs=2) as w1p,
            tc.tile_pool(name="w2p", bufs=2) as w2p,
            tc.tile_pool(name="outp", bufs=2) as outp,
            tc.tile_pool(name="psum", bufs=3, space="PSUM") as psum_pool,
            tc.tile_pool(name="psum3", bufs=1, space="PSUM") as psum3_pool,
        ):
            sc = consts.tile([128, 53], F32)
            nc.sync.dma_start(out=sc, in_=scal[:, :])

            st = acts.tile([D, BL], F32, tag="state")
            nc.sync.dma_start(out=st, in_=stateT[:, :])

            w3sb = consts.tile([128, K3, ACT_DIM], F16, tag="w3")
            nc.sync.dma_start(out=w3sb, in_=w3t[:, :, :])

            # ---- encoder: spine k-tiles = sigmoid(state*scale_k + bias_k)
            spine = []
            for k in range(K1):
                sp = acts.tile([128, BL], F16, tag=f"spine{k}")
                nc.scalar.activation(
                    sp, st, mybir.ActivationFunctionType.Sigmoid,
                    bias=sc[:, 10 + k:11 + k], scale=sc[:, k:k + 1])
                spine.append(sp)

            # ---- layer 1: h1[m] = relu(W1p[m-block] @ spine + b1)
            h1 = []
            for m in range(M1):
                w1sb = w1p.tile([128, K1 * 128], F16, tag="w1")
                nc.sync.dma_start(
                    out=w1sb, in_=w1t[m].rearrange("p k j -> p (k j)"))
                h1m = acts.tile([128, BL], F16, tag=f"h1_{m}")
                for n in range(NSUB):
                    ps = psum_pool.tile([128, NT], F32, tag="ps")
                    for k in range(K1):
                        nc.tensor.matmul(
                            ps, w1sb[:, k * 128:(k + 1) * 128],
                            spine[k][:, n * NT:(n + 1) * NT],
                            start=(k == 0), stop=(k == K1 - 1))
                    nc.scalar.activation(
                        h1m[:, n * NT:(n + 1) * NT], ps,
                        mybir.ActivationFunctionType.Relu,
                        bias=sc[:, 20 + m:21 + m])
                h1.append(h1m)

            # ---- layer 2 + folded layer 3 (interleaved so h2 frees early)
            psum3 = [psum3_pool.tile([ACT_DIM, NT], F32, tag=f"p3_{n}",
                                     name=f"p3_{n}")
                     for n in range(NSUB)]
            for m in range(M2):
                w2sb = w2p.tile([128, K2 * 128], F16, tag="w2")
                nc.sync.dma_start(
                    out=w2sb, in_=w2t[m].rearrange("p k j -> p (k j)"))
                h2m = h2p.tile([128, BL], F16, tag="h2")
                for n in range(NSUB):
                    ps = psum_pool.tile([128, NT], F32, tag="ps")
                    for k in range(K2):
                        nc.tensor.matmul(
                            ps, w2sb[:, k * 128:(k + 1) * 128],
                            h1[k][:, n * NT:(n + 1) * NT],
                            start=(k == 0), stop=(k == K2 - 1))
                    nc.scalar.activation(
                        h2m[:, n * NT:(n + 1) * NT], ps,
                        mybir.ActivationFunctionType.Relu,
                        bias=sc[:, 36 + m:37 + m])
                    # layer 3: accumulate this h2 m-tile into the [32, NT] raw
                    nc.tensor.matmul(
                        psum3[n], w3sb[:, m, :],
                        h2m[:, n * NT:(n + 1) * NT],
                        start=(m == 0), stop=(m == M2 - 1),
                        skip_group_check=True)

            # ---- output: tanh(raw + b3p)
            for n in range(NSUB):
                ot = outp.tile([ACT_DIM, NT], F32, tag="ot")
                nc.scalar.activation(
                    ot, psum3[n], mybir.ActivationFunctionType.Tanh,
                    bias=sc[:ACT_DIM, 52:53])
                nc.sync.dma_start(out=out[:, n * NT:(n + 1) * NT], in_=ot)

    nc.compile()
    _cached["nc"] = nc
    return nc


def _prep_inputs(state, mean_enc, std_enc, W1, b1, W2, b2, W3, b3, Wd, bd):
    f32 = np.float32
    state = np.asarray(state, f32)
    mean_enc = np.asarray(mean_enc, f32)
    std_enc = np.asarray(std_enc, f32)
    W1 = np.asarray(W1, f32)
    b1 = np.asarray(b1, f32)
    W2 = np.asarray(W2, f32)
    b2 = np.asarray(b2, f32)
    W3 = np.asarray(W3, f32)
    b3 = np.asarray(b3, f32)
    Wd = np.asarray(Wd, f32)
    bd = np.asarray(bd, f32)

    # Fold decoder grouped conv into layer 3
    wd = Wd[:, 0, :]                                   # [32, 10]
    W3p = np.einsum("ak,akh->ah", wd, W3.reshape(ACT_DIM, DEC_K, H1))
    b3p = (b3.reshape(ACT_DIM, DEC_K) * wd).sum(1) + bd  # [32]

    # Permute encoder contraction: j' = k*128 + d
    W1p = W1.reshape(H0, D, ENC_K).transpose(0, 2, 1).reshape(H0, D * ENC_K)

    # Pre-tiled weight layouts: [m, p, k, j] = lhsT tile stack
    w1t = np.ascontiguousarray(
        W1p.reshape(M1, 128, K1, 128).transpose(0, 3, 2, 1).astype(np.float16))
    w2t = np.ascontiguousarray(
        W2.reshape(M2, 128, K2, 128).transpose(0, 3, 2, 1).astype(np.float16))
    w3t = np.ascontiguousarray(
        W3p.reshape(ACT_DIM, K3, 128).transpose(2, 1, 0).astype(np.float16))

    scal = np.zeros((128, 53), f32)
    scal[:, 0:10] = 1.0 / std_enc[0]                   # enc scale [128, 10]
    scal[:, 10:20] = -mean_enc[0] / std_enc[0]         # enc bias
    scal[:, 20:36] = b1.reshape(M1, 128).T
    scal[:, 36:52] = b2.reshape(M2, 128).T
    scal[:ACT_DIM, 52] = b3p

    in_maps = []
    for c in range(NCORES):
        shard = np.ascontiguousarray(state[c * BL:(c + 1) * BL].T)  # [128, BL]
        in_maps.append({
            "stateT": shard, "w1t": w1t, "w2t": w2t, "w3t": w3t, "scal": scal,
        })
    return in_maps


def kernel(**inputs):
    nc = _build_program()
    in_maps = _prep_inputs(**inputs)
    res = run_bass_kernel_spmd(nc, in_maps, core_ids=list(range(NCORES)))
    out = np.concatenate(
        [res.results[c]["out"].T for c in range(NCORES)], axis=0)
    return np.ascontiguousarray(out.astype(np.float32))


if __name__ == "__main__":
    rng = np.random.default_rng(0)
    state = rng.standard_normal((B, D), dtype=np.float32)
    mean = np.broadcast_to(
        np.linspace(-3, 3, ENC_K, dtype=np.float32), (1, D, ENC_K)).copy()
    std = np.full((1, D, ENC_K), 0.3872983346207417, np.float32)

    def lin(fan_in, fan_out):
        bound = 1 / np.sqrt(fan_in)
        return (rng.uniform(-bound, bound, (fan_out, fan_in)).astype(np.float32),
                rng.uniform(-bound, bound, fan_out).astype(np.float32))

    W1, b1 = lin(D * ENC_K, H0)
    W2, b2 = lin(H0, H1)
    W3, b3 = lin(H1, ACT_DIM * DEC_K)
    Wd = rng.uniform(-0.3, 0.3, (ACT_DIM, 1, DEC_K)).astype(np.float32)
    bd = rng.uniform(-0.3, 0.3, ACT_DIM).astype(np.float32)

    outp = kernel(state=state, mean_enc=mean, std_enc=std, W1=W1, b1=b1,
                  W2=W2, b2=b2, W3=W3, b3=b3, Wd=Wd, bd=bd)

    # numpy reference
    spine = 1 / (1 + np.exp(-(state[:, :, None] - mean) / std))
    a = np.maximum(spine.reshape(B, -1) @ W1.T + b1, 0)
    a = np.maximum(a @ W2.T + b2, 0)
    a = a @ W3.T + b3
    raw = np.einsum("bak,ak->ba", a.reshape(B, ACT_DIM, DEC_K), Wd[:, 0]) + bd
    ref = np.tanh(raw)
    rel = np.linalg.norm(outp - ref) / np.linalg.norm(ref)
    print("rel err:", rel, "max abs diff:", np.abs(outp - ref).max())
